# revision 5
# baseline (speedup 1.0000x reference)
"""Trainium2 Bass kernel for 16-head causal MultiHeadAttention.

Problem shapes (hardcoded): x [4, 2048, 1024], Wq/Wk/Wv [1024, 1024],
Wc [1024, 1024], bc [1024].  Output [4, 2048, 1024].

Sharding: 8 cores = (batch b in 0..3) x (head-group g in 0..1).
Each core computes 8 heads (512 of the 1024 hidden dims) for one batch
element, including its partial c_proj contribution.  The host sums the
two partials per batch and adds the bias.

Per-core kernel (all matmuls fp32r = full PE rate):
  P1a: Q^T, K^T = Wq_g^T @ x_b^T, Wk_g^T @ x_b^T     [512, 2048] each
  P1b: V      = x_b @ Wv_g  (stored with a ones column per head)
  P2:  per (head, q-chunk): S^T = K @ Q^T tiles -> exp (scale 1/8,
       causal mask) -> O^T/denominator accumulate via [V | 1] stationary
       -> normalize rows by 1/denominator
  P3:  partial out = O @ Wc_g   (O^T chunks are the matmul stationaries)
"""

import numpy as np

B, T, C = 4, 2048, 1024
H_PER_CORE = 8       # heads per core
HL = 512             # local head width  (8 heads * 64)
D = 64               # head dim
QC = 512             # q-chunk width (moving free dim)
KCW = 128            # k-chunk width (S^T psum partitions)
NQC = T // QC        # 4
NKC = T // KCW       # 16
N_CORES = 8

_CACHE = {}


def _emit(nc, tc, tile, mybir, io):
    f32, f32r = mybir.dt.float32, mybir.dt.float32r
    Exp = mybir.ActivationFunctionType.Exp
    xT, wq, wk, wv, wc, maskw, ones, out = (
        io["xT"], io["wq"], io["wk"], io["wv"], io["wc"],
        io["maskw"], io["ones"], io["out"],
    )

    from contextlib import ExitStack

    with ExitStack() as ctx:
        persist = ctx.enter_context(tc.tile_pool(name="persist", bufs=1))
        # Q^T / K^T / O^T: [512 rows, 2048 toks] as [128, 4 chunks, 2048]
        qt = persist.tile([128, 4, T], f32r)
        kt = persist.tile([128, 4, T], f32r)
        ot = persist.tile([128, 4, T], f32r)
        # V': [2048 toks, 8 heads x (64 dims + ones col)] as [128, 16, 520]
        vp = persist.tile([128, NKC, H_PER_CORE * (D + 1)], f32r)
        ones_view = vp.rearrange("p mt (h c) -> p mt h c", c=D + 1)[:, :, :, D]
        nc.sync.dma_start(out=ones_view, in_=ones.bitcast(f32r))

        # ---------------- Phase 1a: Q^T, K^T ----------------
        with (
            tc.tile_pool(name="wqk", bufs=1) as wpool,
            tc.tile_pool(name="xtp", bufs=5) as xtp,
            tc.tile_pool(name="ps1", bufs=8, space="PSUM") as ps1,
        ):
            wq_sb = wpool.tile([128, 8, HL], f32r, tag="wq")
            nc.sync.dma_start(
                out=wq_sb, in_=wq.bitcast(f32r).rearrange("(kc p) m -> p kc m", p=128))
            wk_sb = wpool.tile([128, 8, HL], f32r, tag="wk")
            nc.sync.dma_start(
                out=wk_sb, in_=wk.bitcast(f32r).rearrange("(kc p) m -> p kc m", p=128))
            for n in range(NQC):
                xts = []
                for kc2 in range(4):   # tiles of 2 C-chunks each
                    t = xtp.tile([128, 2, QC], f32r, tag="xt")
                    for j in range(2):
                        kc = kc2 * 2 + j
                        nc.sync.dma_start(
                            out=t[:, j, :],
                            in_=xT.bitcast(f32r)[kc * 128:(kc + 1) * 128,
                                                 n * QC:(n + 1) * QC])
                    xts.append(t)

                def xslice(kc):
                    return xts[kc // 2][:, kc % 2, :]

                for mc in range(4):
                    pq = ps1.tile([128, QC], f32, tag="p1")
                    for kc in range(8):
                        nc.tensor.matmul(
                            out=pq[:], lhsT=wq_sb[:, kc, mc * 128:(mc + 1) * 128],
                            rhs=xslice(kc), start=(kc == 0), stop=(kc == 7))
                    nc.scalar.copy(qt[:, mc, n * QC:(n + 1) * QC], pq[:])
                    pk = ps1.tile([128, QC], f32, tag="p1")
                    for kc in range(8):
                        nc.tensor.matmul(
                            out=pk[:], lhsT=wk_sb[:, kc, mc * 128:(mc + 1) * 128],
                            rhs=xslice(kc), start=(kc == 0), stop=(kc == 7))
                    nc.vector.tensor_copy(kt[:, mc, n * QC:(n + 1) * QC], pk[:])

        # ---------------- Phase 1b: V ----------------
        with (
            tc.tile_pool(name="wvp", bufs=1) as wvpool,
            tc.tile_pool(name="xtp2", bufs=5) as xtp2,
            tc.tile_pool(name="ps1b", bufs=8, space="PSUM") as ps1b,
        ):
            wv_sb = wvpool.tile([128, 8, HL], f32r, tag="wv")
            nc.sync.dma_start(
                out=wv_sb, in_=wv.bitcast(f32r).rearrange("(kc p) m -> p kc m", p=128))
            for n in range(NQC):
                xts = []
                for kc2 in range(4):
                    t = xtp2.tile([128, 2, QC], f32r, tag="xt2")
                    for j in range(2):
                        kc = kc2 * 2 + j
                        nc.sync.dma_start(
                            out=t[:, j, :],
                            in_=xT.bitcast(f32r)[kc * 128:(kc + 1) * 128,
                                                 n * QC:(n + 1) * QC])
                    xts.append(t)
                for mt in range(4):
                    gm = n * 4 + mt           # global token chunk (0..15)
                    pv = ps1b.tile([128, HL], f32, tag="pv")
                    for kc in range(8):
                        nc.tensor.matmul(
                            out=pv[:],
                            lhsT=xts[kc // 2][:, kc % 2, mt * 128:(mt + 1) * 128],
                            rhs=wv_sb[:, kc, :], start=(kc == 0), stop=(kc == 7))
                    for h in range(H_PER_CORE):
                        nc.vector.tensor_copy(
                            vp[:, gm, h * (D + 1):h * (D + 1) + D],
                            pv[:, h * D:(h + 1) * D])

        # ---------------- Phase 2: attention ----------------
        with tc.tile_pool(name="wcp", bufs=1) as wcpool:
            # preload Wc during P2 (used in P3)
            wc_sb = wcpool.tile([128, 4, C], f32r)
            nc.sync.dma_start(
                out=wc_sb, in_=wc.bitcast(f32r).rearrange("(kd p) m -> p kd m", p=128))

            with (
                tc.tile_pool(name="mk", bufs=1) as mkpool,
                tc.tile_pool(name="etp", bufs=3) as etp,
                tc.tile_pool(name="smp", bufs=4) as smp,
                tc.tile_pool(name="psw", bufs=2, space="PSUM") as psw,
                tc.tile_pool(name="pso", bufs=2, space="PSUM") as pso,
            ):
                mask_sb = mkpool.tile([128, 4 * QC], f32r)
                nc.sync.dma_start(out=mask_sb, in_=maskw.bitcast(f32r))

                for h in range(H_PER_CORE):
                    r0 = (h % 2) * 64
                    chh = h // 2
                    for qc in range(NQC):
                        K = 4 * qc + 4      # causal k-chunks for this q-chunk
                        po = pso.tile([128, QC], f32, tag="po")
                        nbatches = (K + 2) // 3
                        for bi in range(nbatches):
                            cnt = min(3, K - bi * 3)
                            pw = psw.tile([128, 3, QC], f32, tag="pw")
                            for i in range(cnt):
                                kc = bi * 3 + i
                                nc.tensor.matmul(
                                    out=pw[:, i, :],
                                    lhsT=kt[r0:r0 + 64, chh, kc * 128:(kc + 1) * 128],
                                    rhs=qt[r0:r0 + 64, chh, qc * QC:(qc + 1) * QC],
                                    start=True, stop=True)
                            ew = etp.tile([128, 3, QC], f32r, tag="et")
                            nc.scalar.activation(
                                ew[:, 0:cnt, :], pw[:, 0:cnt, :], Exp, scale=0.125)
                            for i in range(cnt):
                                d = bi * 3 + i - 4 * qc
                                if d >= 0:   # diagonal tile: causal mask
                                    nc.vector.tensor_mul(
                                        ew[:, i, :], ew[:, i, :],
                                        mask_sb[:, d * QC:(d + 1) * QC])
                            for i in range(cnt):
                                kc = bi * 3 + i
                                nc.tensor.matmul(
                                    out=po[0:D + 1, :],
                                    lhsT=vp[:, kc, h * (D + 1):(h + 1) * (D + 1)],
                                    rhs=ew[:, i, :],
                                    start=(kc == 0), stop=(kc == K - 1))
                        # 1/denom broadcast to 64 partitions via a K=1 matmul
                        # (lhsT = a ones row borrowed from the causal mask)
                        recip = smp.tile([1, QC], f32r, tag="recip")
                        with nc.allow_low_precision(reason="f32r is fp32-width"):
                            nc.vector.reciprocal(recip[:], po[D:D + 1, :])
                        rbp = psw.tile([128, 3, QC], f32, tag="pw")
                        nc.tensor.matmul(
                            out=rbp[0:64, 0, :], lhsT=mask_sb[0:1, 0:64],
                            rhs=recip[:], start=True, stop=True)
                        rb = smp.tile([64, QC], f32, tag="rb")
                        nc.vector.tensor_copy(rb[:], rbp[0:64, 0, :])
                        nc.vector.tensor_mul(
                            ot[r0:r0 + 64, chh, qc * QC:(qc + 1) * QC],
                            po[0:64, :], rb[:])

            # ---------------- Phase 3: c_proj partial ----------------
            with (
                tc.tile_pool(name="stp", bufs=4) as stp,
                tc.tile_pool(name="ps3", bufs=4, space="PSUM") as ps3,
            ):
                for mt in range(NKC):
                    for n2 in range(2):
                        pc = ps3.tile([128, QC], f32, tag="pc")
                        for kd in range(4):
                            nc.tensor.matmul(
                                out=pc[:],
                                lhsT=ot[:, kd, mt * 128:(mt + 1) * 128],
                                rhs=wc_sb[:, kd, n2 * QC:(n2 + 1) * QC],
                                start=(kd == 0), stop=(kd == 3))
                        st = stp.tile([128, QC], f32, tag="st")
                        nc.vector.tensor_copy(st[:], pc[:])
                        nc.sync.dma_start(
                            out=out[mt * 128:(mt + 1) * 128,
                                    n2 * QC:(n2 + 1) * QC],
                            in_=st[:])


def build_program():
    """Build and compile the per-core Bass program (cached)."""
    if "nc" in _CACHE:
        return _CACHE["nc"]
    import concourse.bacc as bacc
    import concourse.tile as tile
    from concourse import mybir

    f32 = mybir.dt.float32
    nc = bacc.Bacc("TRN2", target_bir_lowering=False, debug=False,
                   num_devices=N_CORES)
    io = {
        "xT": nc.dram_tensor("xT", [C, T], f32, kind="ExternalInput").ap(),
        "wq": nc.dram_tensor("wq", [C, HL], f32, kind="ExternalInput").ap(),
        "wk": nc.dram_tensor("wk", [C, HL], f32, kind="ExternalInput").ap(),
        "wv": nc.dram_tensor("wv", [C, HL], f32, kind="ExternalInput").ap(),
        "wc": nc.dram_tensor("wc", [HL, C], f32, kind="ExternalInput").ap(),
        "maskw": nc.dram_tensor("maskw", [128, 4 * QC], f32,
                                kind="ExternalInput").ap(),
        "ones": nc.dram_tensor("ones", [128, NKC, H_PER_CORE], f32,
                               kind="ExternalInput").ap(),
        "out": nc.dram_tensor("out", [T, C], f32, kind="ExternalOutput").ap(),
    }
    with tile.TileContext(nc) as tc:
        _emit(nc, tc, tile, mybir, io)
    nc.compile()
    _CACHE["nc"] = nc
    return nc


def make_in_maps(x, Wq, Wk, Wv, Wc):
    x = np.asarray(x, dtype=np.float32)
    Wq = np.asarray(Wq, dtype=np.float32)
    Wk = np.asarray(Wk, dtype=np.float32)
    Wv = np.asarray(Wv, dtype=np.float32)
    Wc = np.asarray(Wc, dtype=np.float32)

    # maskw[i, d*512 + j] = 1.0 iff j >= i + 128*d   (diagonal tile d)
    i_idx = np.arange(128)[:, None]
    j_idx = np.arange(QC)[None, :]
    maskw = np.concatenate(
        [(j_idx >= i_idx + 128 * d) for d in range(4)], axis=1
    ).astype(np.float32)
    ones = np.ones((128, NKC, H_PER_CORE), dtype=np.float32)

    in_maps = []
    for b in range(B):
        xT = np.ascontiguousarray(x[b].T)
        for g in range(2):
            sl = slice(g * HL, (g + 1) * HL)
            in_maps.append({
                "xT": xT,
                "wq": np.ascontiguousarray(Wq[:, sl]),
                "wk": np.ascontiguousarray(Wk[:, sl]),
                "wv": np.ascontiguousarray(Wv[:, sl]),
                "wc": np.ascontiguousarray(Wc[sl, :]),
                "maskw": maskw,
                "ones": ones,
            })
    return in_maps


def kernel(x, Wq, Wk, Wv, Wc, bc):
    from concourse.bass_utils import run_bass_kernel_spmd

    nc = build_program()
    in_maps = make_in_maps(x, Wq, Wk, Wv, Wc)
    res = run_bass_kernel_spmd(nc, in_maps, core_ids=list(range(N_CORES)))
    bc = np.asarray(bc, dtype=np.float32)
    out = np.empty((B, T, C), dtype=np.float32)
    for b in range(B):
        out[b] = res.results[2 * b]["out"] + res.results[2 * b + 1]["out"] + bc
    return out


# revision 21
# speedup vs baseline: 11.8210x; 11.8210x over previous
"""Trainium2 Bass kernel for 16-head causal MultiHeadAttention.

Problem shapes (hardcoded): x [4, 2048, 1024], Wq/Wk/Wv [1024, 1024],
Wc [1024, 1024], bc [1024].  Output [4, 2048, 1024].

Sharding: 8 cores = (batch b in 0..3) x (head-group g in 0..1).
Each core computes 8 heads (512 of the 1024 hidden dims) for one batch
element, including its partial c_proj contribution.  The host sums the
two partials per batch and adds the bias.

Per-core kernel (all matmuls fp32r = full PE rate):
  P1a: Q^T, K^T = Wq_g^T @ x_b^T, Wk_g^T @ x_b^T    [512, 2048] each
  P1b: V = x_b @ Wv_g  (stored with a ones column per head)
  P2:  per (head, q-chunk): S^T = K @ Q^T tiles -> exp (scale 1/8,
       causal mask) -> O^T/denominator accumulate via [V | 1] stationary
       -> normalize rows by 1/denominator (DRAM-bounce broadcast of the
       denominator + fast reciprocal, off the PSUM critical path)
  P3:  partial out = O @ Wc_g   (O^T chunks are the matmul stationaries)
"""

import numpy as np

B, T, C = 4, 2048, 1024
H_PER_CORE = 8       # heads per core
HL = 512             # local head width  (8 heads * 64)
D = 64               # head dim
QC = 512             # q-chunk width (moving free dim)
NQC = T // QC        # 4
NKC = T // 128       # 16
N_CORES = 8

_CACHE = {}


def _emit(nc, tc, tile, mybir, io):
    import concourse.bass as bass
    f32, f32r = mybir.dt.float32, mybir.dt.float32r
    Exp = mybir.ActivationFunctionType.Exp
    xT, wq, wk, wv, wc, maskw, ones, out = (
        io["xT"], io["wq"], io["wk"], io["wv"], io["wc"],
        io["maskw"], io["ones"], io["out"],
    )

    from contextlib import ExitStack

    with ExitStack() as ctx:
        persist = ctx.enter_context(tc.tile_pool(name="persist", bufs=1))
        # Q^T / K^T / O^T: [512 rows, 2048 toks] as [128, 4 chunks, 2048]
        qt = persist.tile([128, 4, T], f32r)
        kt = persist.tile([128, 4, T], f32r)
        ot = persist.tile([128, 4, T], f32r)
        # V': [2048 toks, 8 heads x (64 dims + ones col)] as [128, 16, 520]
        vp = persist.tile([128, NKC, H_PER_CORE * (D + 1)], f32r)
        ones_view = vp.rearrange("p mt (h c) -> p mt h c", c=D + 1)[:, :, :, D]
        nc.sync.dma_start(out=ones_view, in_=ones.bitcast(f32r))

        def load_xt(pool, tag, n):
            """x^T [1024, 512-tok chunk n] as 4 tiles of 2 C-chunks."""
            xts = []
            for kc2 in range(4):
                t = pool.tile([128, 2, QC], f32r, tag=tag)
                for j in range(2):
                    kc = kc2 * 2 + j
                    nc.sync.dma_start(
                        out=t[:, j, :],
                        in_=xT.bitcast(f32r)[kc * 128:(kc + 1) * 128,
                                             n * QC:(n + 1) * QC])
                xts.append(t)
            return lambda kc: xts[kc // 2][:, kc % 2, :]

        # ---------------- Phase 1a: Q^T, K^T ----------------
        with (
            tc.tile_pool(name="wqk", bufs=1) as wpool,
            tc.tile_pool(name="xtp", bufs=5) as xtp,
            tc.tile_pool(name="ps1", bufs=8, space="PSUM") as ps1,
        ):
            wq_sb = wpool.tile([128, 8, HL], f32r, tag="wq")
            wk_sb = wpool.tile([128, 8, HL], f32r, tag="wk")
            for kc in range(8):
                nc.sync.dma_start(
                    out=wq_sb[:, kc, :],
                    in_=wq.bitcast(f32r)[kc * 128:(kc + 1) * 128, :])
                nc.sync.dma_start(
                    out=wk_sb[:, kc, :],
                    in_=wk.bitcast(f32r)[kc * 128:(kc + 1) * 128, :])
            for n in range(NQC):
                xs = load_xt(xtp, "xt", n)
                for mc in range(4):
                    pq = ps1.tile([128, QC], f32, tag="p1")
                    for kc in range(8):
                        nc.tensor.matmul(
                            out=pq[:], lhsT=wq_sb[:, kc, mc * 128:(mc + 1) * 128],
                            rhs=xs(kc), start=(kc == 0), stop=(kc == 7))
                    nc.scalar.copy(qt[:, mc, n * QC:(n + 1) * QC], pq[:])
                    pk = ps1.tile([128, QC], f32, tag="p1")
                    for kc in range(8):
                        nc.tensor.matmul(
                            out=pk[:], lhsT=wk_sb[:, kc, mc * 128:(mc + 1) * 128],
                            rhs=xs(kc), start=(kc == 0), stop=(kc == 7))
                    nc.vector.tensor_copy(kt[:, mc, n * QC:(n + 1) * QC], pk[:])

        # ---------------- Phase 1b: V ----------------
        with (
            tc.tile_pool(name="wvp", bufs=1) as wvpool,
            tc.tile_pool(name="xtp2", bufs=5) as xtp2,
            tc.tile_pool(name="ps1b", bufs=8, space="PSUM") as ps1b,
        ):
            wv_sb = wvpool.tile([128, 8, HL], f32r, tag="wv")
            for kc in range(8):
                nc.sync.dma_start(
                    out=wv_sb[:, kc, :],
                    in_=wv.bitcast(f32r)[kc * 128:(kc + 1) * 128, :])
            for n in range(NQC):
                xs = load_xt(xtp2, "xt2", n)
                for mt in range(4):
                    gm = n * 4 + mt           # global token chunk (0..15)
                    pv = ps1b.tile([128, HL], f32, tag="pv")
                    for kc in range(8):
                        nc.tensor.matmul(
                            out=pv[:], lhsT=xs(kc)[:, mt * 128:(mt + 1) * 128],
                            rhs=wv_sb[:, kc, :], start=(kc == 0), stop=(kc == 7))
                    for h in range(H_PER_CORE):
                        nc.vector.tensor_copy(
                            vp[:, gm, h * (D + 1):h * (D + 1) + D],
                            pv[:, h * D:(h + 1) * D])

        # ---------------- Phase 2: attention ----------------
        with tc.tile_pool(name="wcp", bufs=1) as wcpool:
            # preload Wc during P2 (used in P3)
            wc_sb = wcpool.tile([128, 4, C], f32r)
            nc.sync.dma_start(
                out=wc_sb, in_=wc.bitcast(f32r).rearrange("(kd p) m -> p kd m", p=128))

            with (
                tc.tile_pool(name="mk", bufs=1) as mkpool,
                tc.tile_pool(name="etp", bufs=3) as etp,
                tc.tile_pool(name="smp", bufs=4) as smp,
                tc.tile_pool(name="drp", bufs=4, space="DRAM") as drp,
                tc.tile_pool(name="psw", bufs=2, space="PSUM") as psw,
                tc.tile_pool(name="pso", bufs=2, space="PSUM") as pso,
            ):
                mask_sb = mkpool.tile([128, 4 * QC], f32r)
                nc.sync.dma_start(out=mask_sb, in_=maskw.bitcast(f32r))

                for h in range(H_PER_CORE):
                    r0 = (h % 2) * 64
                    chh = h // 2
                    for qc in range(NQC):
                        K = 4 * qc + 4      # causal k-chunks for this q-chunk
                        po = pso.tile([128, QC], f32, tag="po")
                        nbatches = (K + 2) // 3
                        for bi in range(nbatches):
                            cnt = min(3, K - bi * 3)
                            pw = psw.tile([128, 3, QC], f32, tag="pw")
                            for i in range(cnt):
                                kc = bi * 3 + i
                                nc.tensor.matmul(
                                    out=pw[:, i, :],
                                    lhsT=kt[r0:r0 + 64, chh, kc * 128:(kc + 1) * 128],
                                    rhs=qt[r0:r0 + 64, chh, qc * QC:(qc + 1) * QC],
                                    start=True, stop=True)
                            ew = etp.tile([128, 3, QC], f32r, tag="et")
                            nc.scalar.activation(
                                ew[:, 0:cnt, :], pw[:, 0:cnt, :], Exp, scale=0.125)
                            for i in range(cnt):
                                d = bi * 3 + i - 4 * qc
                                if d >= 0:   # diagonal tile: causal mask
                                    nc.vector.tensor_mul(
                                        ew[:, i, :], ew[:, i, :],
                                        mask_sb[:, d * QC:(d + 1) * QC])
                            for i in range(cnt):
                                kc = bi * 3 + i
                                nc.tensor.matmul(
                                    out=po[0:D + 1, :],
                                    lhsT=vp[:, kc, h * (D + 1):(h + 1) * (D + 1)],
                                    rhs=ew[:, i, :],
                                    start=(kc == 0), stop=(kc == K - 1))
                        # Evict PSUM fast (2 copies); normalize off the
                        # critical path: denominator -> DRAM bounce ->
                        # partition-broadcast DMA -> fast reciprocal ->
                        # in-place scale of O^T.
                        ot_slice = ot[r0:r0 + 64, chh, qc * QC:(qc + 1) * QC]
                        nc.vector.tensor_copy(ot_slice, po[0:64, :])
                        d1 = smp.tile([1, QC], f32, tag="d1")
                        nc.vector.tensor_copy(d1[:], po[D:D + 1, :])
                        scr = drp.tile([1, QC], f32, tag="scr")
                        nc.sync.dma_start(out=scr[:], in_=d1[:])
                        db = smp.tile([128, QC], f32, tag="db")
                        s0 = scr[:]
                        # broadcast to all 128 partitions: the custom-DVE
                        # reciprocal is only correct at base partition 0.
                        nc.gpsimd.dma_start(
                            out=db[:],
                            in_=bass.AP(tensor=s0.tensor, offset=s0.offset,
                                        ap=[[0, 128], [1, QC]]))
                        nc.vector.reciprocal_approx_fast(db[:], db[:])
                        nc.vector.tensor_mul(ot_slice, ot_slice, db[r0:r0 + 64, :])

            # ---------------- Phase 3: c_proj partial ----------------
            with (
                tc.tile_pool(name="stp", bufs=4) as stp,
                tc.tile_pool(name="ps3", bufs=4, space="PSUM") as ps3,
            ):
                for mt in range(NKC):
                    for n2 in range(2):
                        pc = ps3.tile([128, QC], f32, tag="pc")
                        for kd in range(4):
                            nc.tensor.matmul(
                                out=pc[:],
                                lhsT=ot[:, kd, mt * 128:(mt + 1) * 128],
                                rhs=wc_sb[:, kd, n2 * QC:(n2 + 1) * QC],
                                start=(kd == 0), stop=(kd == 3))
                        st = stp.tile([128, QC], f32, tag="st")
                        nc.vector.tensor_copy(st[:], pc[:])
                        nc.sync.dma_start(
                            out=out[mt * 128:(mt + 1) * 128,
                                    n2 * QC:(n2 + 1) * QC],
                            in_=st[:])


def build_program():
    """Build and compile the per-core Bass program (cached)."""
    if "nc" in _CACHE:
        return _CACHE["nc"]
    import concourse.bacc as bacc
    import concourse.tile as tile
    from concourse import mybir

    f32 = mybir.dt.float32
    nc = bacc.Bacc("TRN2", target_bir_lowering=False, debug=False,
                   num_devices=N_CORES)
    io = {
        "xT": nc.dram_tensor("xT", [C, T], f32, kind="ExternalInput").ap(),
        "wq": nc.dram_tensor("wq", [C, HL], f32, kind="ExternalInput").ap(),
        "wk": nc.dram_tensor("wk", [C, HL], f32, kind="ExternalInput").ap(),
        "wv": nc.dram_tensor("wv", [C, HL], f32, kind="ExternalInput").ap(),
        "wc": nc.dram_tensor("wc", [HL, C], f32, kind="ExternalInput").ap(),
        "maskw": nc.dram_tensor("maskw", [128, 4 * QC], f32,
                                kind="ExternalInput").ap(),
        "ones": nc.dram_tensor("ones", [128, NKC, H_PER_CORE], f32,
                               kind="ExternalInput").ap(),
        "out": nc.dram_tensor("out", [T, C], f32, kind="ExternalOutput").ap(),
    }
    with tile.TileContext(nc) as tc:
        _emit(nc, tc, tile, mybir, io)
    nc.compile()
    _CACHE["nc"] = nc
    return nc


def make_in_maps(x, Wq, Wk, Wv, Wc):
    x = np.asarray(x, dtype=np.float32)
    Wq = np.asarray(Wq, dtype=np.float32)
    Wk = np.asarray(Wk, dtype=np.float32)
    Wv = np.asarray(Wv, dtype=np.float32)
    Wc = np.asarray(Wc, dtype=np.float32)

    # maskw[i, d*512 + j] = 1.0 iff j >= i + 128*d   (diagonal tile d)
    i_idx = np.arange(128)[:, None]
    j_idx = np.arange(QC)[None, :]
    maskw = np.concatenate(
        [(j_idx >= i_idx + 128 * d) for d in range(4)], axis=1
    ).astype(np.float32)
    ones = np.ones((128, NKC, H_PER_CORE), dtype=np.float32)

    in_maps = []
    for b in range(B):
        xT = np.ascontiguousarray(x[b].T)
        for g in range(2):
            sl = slice(g * HL, (g + 1) * HL)
            in_maps.append({
                "xT": xT,
                "wq": np.ascontiguousarray(Wq[:, sl]),
                "wk": np.ascontiguousarray(Wk[:, sl]),
                "wv": np.ascontiguousarray(Wv[:, sl]),
                "wc": np.ascontiguousarray(Wc[sl, :]),
                "maskw": maskw,
                "ones": ones,
            })
    return in_maps


def kernel(x, Wq, Wk, Wv, Wc, bc):
    from concourse.bass_utils import run_bass_kernel_spmd

    nc = build_program()
    in_maps = make_in_maps(x, Wq, Wk, Wv, Wc)
    res = run_bass_kernel_spmd(nc, in_maps, core_ids=list(range(N_CORES)))
    bc = np.asarray(bc, dtype=np.float32)
    out = np.empty((B, T, C), dtype=np.float32)
    for b in range(B):
        out[b] = res.results[2 * b]["out"] + res.results[2 * b + 1]["out"] + bc
    return out


# revision 29
# speedup vs baseline: 11.9230x; 1.0086x over previous
"""Trainium2 Bass kernel for 16-head causal MultiHeadAttention.

Problem shapes (hardcoded): x [4, 2048, 1024], Wq/Wk/Wv [1024, 1024],
Wc [1024, 1024], bc [1024].  Output [4, 2048, 1024].

Sharding: 8 cores = (batch b in 0..3) x (head-group g in 0..1).
Each core computes 8 heads (512 of the 1024 hidden dims) for one batch
element, including its partial c_proj contribution.  The host sums the
two partials per batch and adds the bias.

Per-core kernel (all matmuls fp32r = full PE rate):
  P1:  one pass over x^T computing Q^T, K^T = W^T @ x_b^T [512, 2048]
       and V = x_b @ Wv_g (stored with a ones column per head)
  P2:  per (head, q-chunk): S^T = K @ Q^T tiles -> exp (scale 1/8,
       causal mask) -> O^T/denominator accumulate via [V | 1] stationary
       -> normalize rows by 1/denominator (DRAM-bounce broadcast of the
       denominator + fast reciprocal, off the PSUM critical path)
  P3:  partial out = O @ Wc_g   (O^T chunks are the matmul stationaries)
"""

import numpy as np

B, T, C = 4, 2048, 1024
H_PER_CORE = 8       # heads per core
HL = 512             # local head width  (8 heads * 64)
D = 64               # head dim
QC = 512             # q-chunk width (moving free dim)
NQC = T // QC        # 4
NKC = T // 128       # 16
N_CORES = 8

_CACHE = {}


def _emit(nc, tc, tile, mybir, io):
    import concourse.bass as bass
    f32, f32r = mybir.dt.float32, mybir.dt.float32r
    Exp = mybir.ActivationFunctionType.Exp
    xT, wq, wk, wv, wc, maskw, ones, out = (
        io["xT"], io["wq"], io["wk"], io["wv"], io["wc"],
        io["maskw"], io["ones"], io["out"],
    )

    from contextlib import ExitStack

    with ExitStack() as ctx:
        persist = ctx.enter_context(tc.tile_pool(name="persist", bufs=1))
        # Q^T / K^T / O^T: [512 rows, 2048 toks] as [128, 4 chunks, 2048]
        qt = persist.tile([128, 4, T], f32r)
        kt = persist.tile([128, 4, T], f32r)
        # V': [2048 toks, 8 heads x (64 dims + ones col)] as [128, 16, 520]
        vp = persist.tile([128, NKC, H_PER_CORE * (D + 1)], f32r)
        ones_view = vp.rearrange("p mt (h c) -> p mt h c", c=D + 1)[:, :, :, D]
        nc.sync.dma_start(out=ones_view, in_=ones.bitcast(f32r))

        def load_xt(pool, tag, n):
            """x^T [1024, 512-tok chunk n] as 4 tiles of 2 C-chunks."""
            xts = []
            for kc2 in range(4):
                t = pool.tile([128, 2, QC], f32r, tag=tag)
                for j in range(2):
                    kc = kc2 * 2 + j
                    nc.sync.dma_start(
                        out=t[:, j, :],
                        in_=xT.bitcast(f32r)[kc * 128:(kc + 1) * 128,
                                             n * QC:(n + 1) * QC])
                xts.append(t)
            return lambda kc: xts[kc // 2][:, kc % 2, :]

        # ------- Phase 1: Q^T, K^T, V in one pass over x^T -------
        with (
            tc.tile_pool(name="wqk", bufs=1) as wpool,
            tc.tile_pool(name="xtp", bufs=5) as xtp,
            tc.tile_pool(name="ps1", bufs=8, space="PSUM") as ps1,
        ):
            wq_sb = wpool.tile([128, 8, HL], f32r, tag="wq")
            wk_sb = wpool.tile([128, 8, HL], f32r, tag="wk")
            wv_sb = wpool.tile([128, 8, HL], f32r, tag="wv")
            for kc in range(8):
                nc.sync.dma_start(
                    out=wq_sb[:, kc, :],
                    in_=wq.bitcast(f32r)[kc * 128:(kc + 1) * 128, :])
                nc.sync.dma_start(
                    out=wk_sb[:, kc, :],
                    in_=wk.bitcast(f32r)[kc * 128:(kc + 1) * 128, :])
                nc.sync.dma_start(
                    out=wv_sb[:, kc, :],
                    in_=wv.bitcast(f32r)[kc * 128:(kc + 1) * 128, :])
            for n in range(NQC):
                xs = load_xt(xtp, "xt", n)
                for mc in range(4):
                    pq = ps1.tile([128, QC], f32, tag="p1")
                    for kc in range(8):
                        nc.tensor.matmul(
                            out=pq[:], lhsT=wq_sb[:, kc, mc * 128:(mc + 1) * 128],
                            rhs=xs(kc), start=(kc == 0), stop=(kc == 7))
                    nc.scalar.copy(qt[:, mc, n * QC:(n + 1) * QC], pq[:])
                    pk = ps1.tile([128, QC], f32, tag="p1")
                    for kc in range(8):
                        nc.tensor.matmul(
                            out=pk[:], lhsT=wk_sb[:, kc, mc * 128:(mc + 1) * 128],
                            rhs=xs(kc), start=(kc == 0), stop=(kc == 7))
                    nc.vector.tensor_copy(kt[:, mc, n * QC:(n + 1) * QC], pk[:])
                for mt in range(4):
                    gm = n * 4 + mt           # global token chunk (0..15)
                    pv = ps1.tile([128, HL], f32, tag="p1")
                    for kc in range(8):
                        nc.tensor.matmul(
                            out=pv[:], lhsT=xs(kc)[:, mt * 128:(mt + 1) * 128],
                            rhs=wv_sb[:, kc, :], start=(kc == 0), stop=(kc == 7))
                    for h in range(H_PER_CORE):
                        nc.vector.tensor_copy(
                            vp[:, gm, h * (D + 1):h * (D + 1) + D],
                            pv[:, h * D:(h + 1) * D])

        # ---------------- Phase 2: attention ----------------
        with tc.tile_pool(name="otp", bufs=1) as otpool, \
             tc.tile_pool(name="wcp", bufs=1) as wcpool:
            ot = otpool.tile([128, 4, T], f32r)
            # preload Wc during P2 (used in P3)
            wc_sb = wcpool.tile([128, 4, C], f32r)
            nc.sync.dma_start(
                out=wc_sb, in_=wc.bitcast(f32r).rearrange("(kd p) m -> p kd m", p=128))

            with (
                tc.tile_pool(name="mk", bufs=1) as mkpool,
                tc.tile_pool(name="etp", bufs=3) as etp,
                tc.tile_pool(name="smp", bufs=4) as smp,
                tc.tile_pool(name="drp", bufs=4, space="DRAM") as drp,
                tc.tile_pool(name="psw", bufs=2, space="PSUM") as psw,
                tc.tile_pool(name="pso", bufs=2, space="PSUM") as pso,
            ):
                mask_sb = mkpool.tile([128, 4 * QC], f32r)
                nc.sync.dma_start(out=mask_sb, in_=maskw.bitcast(f32r))

                def emit_batch(h, qc, po, bi, cnt):
                    """S-mms -> exp -> mask -> O-mms for one batch."""
                    r0 = (h % 2) * 64
                    chh = h // 2
                    K = 4 * qc + 4
                    pw = psw.tile([128, 3, QC], f32, tag="pw")
                    for i in range(cnt):
                        kc = bi * 3 + i
                        # 64-row array tiling: even heads use PE rows 0-63,
                        # odd heads rows 64-127 — paired S-matmuls of the
                        # two heads run concurrently on the two halves.
                        nc.tensor.matmul(
                            out=pw[:, i, :],
                            lhsT=kt[r0:r0 + 64, chh, kc * 128:(kc + 1) * 128],
                            rhs=qt[r0:r0 + 64, chh, qc * QC:(qc + 1) * QC],
                            start=True, stop=True, tile_position=(r0, 0))
                    ew = etp.tile([128, 3, QC], f32r, tag="et")
                    nc.scalar.activation(
                        ew[:, 0:cnt, :], pw[:, 0:cnt, :], Exp, scale=0.125)
                    for i in range(cnt):
                        d = bi * 3 + i - 4 * qc
                        if d >= 0:           # diagonal tile: causal mask
                            nc.vector.tensor_mul(
                                ew[:, i, :], ew[:, i, :],
                                mask_sb[:, d * QC:(d + 1) * QC])
                    for i in range(cnt):
                        kc = bi * 3 + i
                        nc.tensor.matmul(
                            out=po[0:D + 1, :],
                            lhsT=vp[:, kc, h * (D + 1):(h + 1) * (D + 1)],
                            rhs=ew[:, i, :],
                            start=(kc == 0), stop=(kc == K - 1))

                def evict(h, qc, po):
                    """Fast PSUM eviction + off-critical-path normalization."""
                    r0 = (h % 2) * 64
                    chh = h // 2
                    ot_slice = ot[r0:r0 + 64, chh, qc * QC:(qc + 1) * QC]
                    nc.vector.tensor_copy(ot_slice, po[0:64, :])
                    d1 = smp.tile([1, QC], f32, tag="d1")
                    nc.vector.tensor_copy(d1[:], po[D:D + 1, :])
                    scr = drp.tile([1, QC], f32, tag="scr")
                    nc.sync.dma_start(out=scr[:], in_=d1[:])
                    db = smp.tile([128, QC], f32, tag="db")
                    s0 = scr[:]
                    # broadcast to all 128 partitions: the custom-DVE
                    # reciprocal is only correct at base partition 0.
                    nc.gpsimd.dma_start(
                        out=db[:],
                        in_=bass.AP(tensor=s0.tensor, offset=s0.offset,
                                    ap=[[0, 128], [1, QC]]))
                    nc.vector.reciprocal_approx_fast(db[:], db[:])
                    nc.vector.tensor_mul(ot_slice, ot_slice, db[r0:r0 + 64, :])

                # Head pairs in batch-lockstep so the two heads' 64-row
                # S-matmuls are adjacent and fill both PE array halves.
                for hp in range(H_PER_CORE // 2):
                    ha, hb = 2 * hp, 2 * hp + 1
                    for qc in range(NQC):
                        K = 4 * qc + 4      # causal k-chunks for this q-chunk
                        po_a = pso.tile([128, QC], f32, tag="po")
                        po_b = pso.tile([128, QC], f32, tag="po")
                        nbatches = (K + 2) // 3
                        for bi in range(nbatches):
                            cnt = min(3, K - bi * 3)
                            emit_batch(ha, qc, po_a, bi, cnt)
                            emit_batch(hb, qc, po_b, bi, cnt)
                        evict(ha, qc, po_a)
                        evict(hb, qc, po_b)

            # ---------------- Phase 3: c_proj partial ----------------
            with (
                tc.tile_pool(name="stp", bufs=4) as stp,
                tc.tile_pool(name="ps3", bufs=4, space="PSUM") as ps3,
            ):
                for mt in range(NKC):
                    for n2 in range(2):
                        pc = ps3.tile([128, QC], f32, tag="pc")
                        for kd in range(4):
                            nc.tensor.matmul(
                                out=pc[:],
                                lhsT=ot[:, kd, mt * 128:(mt + 1) * 128],
                                rhs=wc_sb[:, kd, n2 * QC:(n2 + 1) * QC],
                                start=(kd == 0), stop=(kd == 3))
                        st = stp.tile([128, QC], f32, tag="st")
                        nc.vector.tensor_copy(st[:], pc[:])
                        nc.sync.dma_start(
                            out=out[mt * 128:(mt + 1) * 128,
                                    n2 * QC:(n2 + 1) * QC],
                            in_=st[:])


def build_program():
    """Build and compile the per-core Bass program (cached)."""
    if "nc" in _CACHE:
        return _CACHE["nc"]
    import concourse.bacc as bacc
    import concourse.tile as tile
    from concourse import mybir

    f32 = mybir.dt.float32
    nc = bacc.Bacc("TRN2", target_bir_lowering=False, debug=False,
                   num_devices=N_CORES)
    io = {
        "xT": nc.dram_tensor("xT", [C, T], f32, kind="ExternalInput").ap(),
        "wq": nc.dram_tensor("wq", [C, HL], f32, kind="ExternalInput").ap(),
        "wk": nc.dram_tensor("wk", [C, HL], f32, kind="ExternalInput").ap(),
        "wv": nc.dram_tensor("wv", [C, HL], f32, kind="ExternalInput").ap(),
        "wc": nc.dram_tensor("wc", [HL, C], f32, kind="ExternalInput").ap(),
        "maskw": nc.dram_tensor("maskw", [128, 4 * QC], f32,
                                kind="ExternalInput").ap(),
        "ones": nc.dram_tensor("ones", [128, NKC, H_PER_CORE], f32,
                               kind="ExternalInput").ap(),
        "out": nc.dram_tensor("out", [T, C], f32, kind="ExternalOutput").ap(),
    }
    with tile.TileContext(nc) as tc:
        _emit(nc, tc, tile, mybir, io)
    nc.compile()
    _CACHE["nc"] = nc
    return nc


def make_in_maps(x, Wq, Wk, Wv, Wc):
    x = np.asarray(x, dtype=np.float32)
    Wq = np.asarray(Wq, dtype=np.float32)
    Wk = np.asarray(Wk, dtype=np.float32)
    Wv = np.asarray(Wv, dtype=np.float32)
    Wc = np.asarray(Wc, dtype=np.float32)

    # maskw[i, d*512 + j] = 1.0 iff j >= i + 128*d   (diagonal tile d)
    i_idx = np.arange(128)[:, None]
    j_idx = np.arange(QC)[None, :]
    maskw = np.concatenate(
        [(j_idx >= i_idx + 128 * d) for d in range(4)], axis=1
    ).astype(np.float32)
    ones = np.ones((128, NKC, H_PER_CORE), dtype=np.float32)

    in_maps = []
    for b in range(B):
        xT = np.ascontiguousarray(x[b].T)
        for g in range(2):
            sl = slice(g * HL, (g + 1) * HL)
            in_maps.append({
                "xT": xT,
                "wq": np.ascontiguousarray(Wq[:, sl]),
                "wk": np.ascontiguousarray(Wk[:, sl]),
                "wv": np.ascontiguousarray(Wv[:, sl]),
                "wc": np.ascontiguousarray(Wc[sl, :]),
                "maskw": maskw,
                "ones": ones,
            })
    return in_maps


def kernel(x, Wq, Wk, Wv, Wc, bc):
    from concourse.bass_utils import run_bass_kernel_spmd

    nc = build_program()
    in_maps = make_in_maps(x, Wq, Wk, Wv, Wc)
    res = run_bass_kernel_spmd(nc, in_maps, core_ids=list(range(N_CORES)))
    bc = np.asarray(bc, dtype=np.float32)
    out = np.empty((B, T, C), dtype=np.float32)
    for b in range(B):
        out[b] = res.results[2 * b]["out"] + res.results[2 * b + 1]["out"] + bc
    return out
